# revision 6
# baseline (speedup 1.0000x reference)
"""ACMIL top-k masking kernel for 8 TRN2 NeuronCores.

Reference computation (N=50000, D=1024, BRANCHES=4, TOP_K=10):
    scores = features @ W.T + b          # [N, 4]   (b cancels in softmax)
    weights = softmax(scores, axis=0)    # over instances
    w = weights.mean(axis=1)             # [N]
    w[top_k(w, 10)] = 0
    w = softmax(w, axis=0)
    bag = w @ features                   # [D]
    returns (bag, w)

Distribution: shard instances (dim 0) across 8 cores (6250 rows each).
AllReduce the per-branch softmax normalizer, global top-k via per-shard
top-16 + AllGather + local rank-10 threshold, AllReduce the final pooled
bag + second-softmax denominator.
"""

import sys

for _p in ("/opt/trn_rl_repo",):
    if _p not in sys.path:
        sys.path.insert(0, _p)

import numpy as np
import ml_dtypes

import concourse.bass as bass
import concourse.bacc as bacc
import concourse.mybir as mybir
import concourse.tile as tile
from concourse.bass_utils import run_bass_kernel_spmd

F32 = mybir.dt.float32
BF16 = mybir.dt.bfloat16
I32 = mybir.dt.int32

N, D, BR, TOPK, CORES = 50000, 1024, 4, 10, 8
NS = N // CORES  # 6250 rows per core


def build_nc(ns=NS, d=D, br=BR, cores=CORES, use_dma_cast=True):
    """Build the per-core Bass graph (SPMD: same graph on all cores)."""
    P = 128
    T = (ns + P - 1) // P          # n-tiles per shard
    PROWS = ns - (T - 1) * P       # real rows in last tile
    NPAD_L = T * P - ns            # pad rows per core
    DC = d // P                    # d-chunks
    topk = TOPK

    nc = bacc.Bacc("TRN2", target_bir_lowering=False, debug=False,
                   num_devices=cores)

    feats = nc.dram_tensor("features", [ns, d], F32, kind="ExternalInput").ap()
    # host passes W.T pre-cast to bf16 (4KB, layout prep only)
    wt_in = nc.dram_tensor("wt", [d, br], BF16, kind="ExternalInput").ap()
    out_w = nc.dram_tensor("out_w", [ns], F32, kind="ExternalOutput").ap()
    out_bag = nc.dram_tensor("out_bag", [d], F32, kind="ExternalOutput").ap()

    rg = [list(range(cores))]

    with tile.TileContext(nc) as tc:
        with (
            tc.tile_pool(name="fb", bufs=1) as fbp,       # resident bf16 tiles
            tc.tile_pool(name="ft", bufs=4) as ftp,       # fT stream
            tc.tile_pool(name="sm", bufs=1) as smp,       # small persistents
            tc.tile_pool(name="ps", bufs=1, space="PSUM") as psp,
            tc.tile_pool(name="dr", bufs=1, space="DRAM") as drp,
        ):
            # ---------------- setup ----------------
            wt_sb = smp.tile([P, DC * br], BF16, tag="wt_sb")
            nc.sync.dma_start(
                out=wt_sb[:, :].rearrange("p (c b) -> p c b", b=br),
                in_=wt_in.rearrange("(c p) b -> p c b", p=P),
            )
            ones_bf = smp.tile([P, 1], BF16, tag="ones_bf")
            nc.gpsimd.memset(ones_bf[:, :], 1.0)
            ones_f32 = smp.tile([P, 1], F32, tag="ones_f32")
            nc.gpsimd.memset(ones_f32[:, :], 1.0)

            # identity matrix for PE transpose of the output weights
            iota_j = smp.tile([P, P], I32, tag="iota_j")
            nc.gpsimd.iota(iota_j[:, :], pattern=[[1, P]], base=0,
                           channel_multiplier=0)
            iota_p = smp.tile([P, 1], I32, tag="iota_p")
            nc.gpsimd.iota(iota_p[:, :], pattern=[[0, 1]], base=0,
                           channel_multiplier=1)
            iota_jf = smp.tile([P, P], F32, tag="iota_jf")
            nc.vector.tensor_copy(iota_jf[:, :], iota_j[:, :])
            iota_pf = smp.tile([P, 1], F32, tag="iota_pf")
            nc.vector.tensor_copy(iota_pf[:, :], iota_p[:, :])
            ident = smp.tile([P, P], F32, tag="ident")
            nc.vector.tensor_scalar(ident[:, :], iota_jf[:, :],
                                    iota_pf[:, :], None,
                                    mybir.AluOpType.is_equal)

            # ---------------- phase A: load + scores + bag0 ----------------
            ps_sc = psp.tile([P, T * br], F32, tag="ps_sc")    # score accum
            ps_b0 = psp.tile([1, d], F32, tag="ps_b0")         # sum_n f[n,:]

            fb = []
            for t in range(T):
                fb_t = fbp.tile([P, d], BF16, tag=f"fb{t}")
                fb.append(fb_t)
                rows = PROWS if t == T - 1 else P
                if rows < P:
                    nc.gpsimd.memset(fb_t[:, :], 0.0)
                if use_dma_cast:
                    nc.gpsimd.dma_start(out=fb_t[0:rows, :],
                                        in_=feats[t * P: t * P + rows, :])
                else:
                    fs_t = ftp.tile([P, d], F32, tag="fs")
                    nc.sync.dma_start(out=fs_t[0:rows, :],
                                      in_=feats[t * P: t * P + rows, :])
                    nc.scalar.copy(fb_t[0:rows, :], fs_t[0:rows, :])

                ft_t = ftp.tile([P, d], BF16, tag="ft")
                nc.sync.dma_start_transpose(
                    out=ft_t[:, :].rearrange("p (c j) -> p c j", c=DC),
                    in_=fb_t[:, :],
                )
                for c in range(DC):
                    nc.tensor.matmul(
                        ps_sc[:, t * br:(t + 1) * br],
                        lhsT=ft_t[:, c * P:(c + 1) * P],
                        rhs=wt_sb[:, c * br:(c + 1) * br],
                        start=(c == 0), stop=(c == DC - 1),
                    )
                for h0 in range(0, d, 512):
                    h1 = min(h0 + 512, d)
                    nc.tensor.matmul(
                        ps_b0[0:1, h0:h1],
                        lhsT=ones_bf[:, :],
                        rhs=fb_t[:, h0:h1],
                        start=(t == 0), stop=(t == T - 1),
                        skip_group_check=True,
                    )

            # ---------------- e = exp(scores), branch sums ----------------
            e_sb = smp.tile([P, T * br], F32, tag="e_sb")
            nc.scalar.activation(e_sb[:, :], ps_sc[:, :],
                                 mybir.ActivationFunctionType.Exp)
            sp = smp.tile([P, br], F32, tag="sp")  # per-partition branch sums
            nc.vector.tensor_reduce(
                sp[:, :],
                e_sb[:, :].rearrange("p (t b) -> p b t", b=br),
                axis=mybir.AxisListType.X, op=mybir.AluOpType.add,
            )
            ps_s = psp.tile([1, br], F32, tag="ps_s")
            nc.tensor.matmul(ps_s[0:1, :], lhsT=ones_f32[:, :], rhs=sp[:, :],
                             start=True, stop=True)
            s_row = smp.tile([1, br], F32, tag="s_row")
            nc.scalar.copy(s_row[:, :], ps_s[0:1, :])

            # R1: AllReduce branch normalizers
            cc1_in = drp.tile([1, br], F32, tag="cc1_in")
            cc1_out = drp.tile([1, br], F32, tag="cc1_out")
            nc.sync.dma_start(out=cc1_in[:, :], in_=s_row[:, :])
            nc.gpsimd.collective_compute(
                "AllReduce", mybir.AluOpType.add, replica_groups=rg,
                ins=[cc1_in[:, :].opt()], outs=[cc1_out[:, :].opt()],
            )
            sg = smp.tile([1, br], F32, tag="sg")
            nc.sync.dma_start(out=sg[:, :], in_=cc1_out[:, :])
            if NPAD_L > 0:
                nc.vector.tensor_scalar_add(sg[:, :], sg[:, :],
                                            float(-NPAD_L * cores))

            # w[n] = sum_br e[n,br] / (4*S_br)   -> [128, T]
            rs = smp.tile([1, br], F32, tag="rs")
            nc.vector.reciprocal(rs[:, :], sg[:, :])
            nc.vector.tensor_scalar_mul(rs[:, :], rs[:, :], 1.0 / br)
            rs_bc = smp.tile([P, br], F32, tag="rs_bc")
            nc.gpsimd.partition_broadcast(rs_bc[:, :], rs[:, :])
            w4 = smp.tile([P, T * br], F32, tag="w4")
            e3 = e_sb[:, :].rearrange("p (t b) -> p t b", b=br)
            w43 = w4[:, :].rearrange("p (t b) -> p t b", b=br)
            for j in range(br):
                nc.vector.tensor_scalar(
                    w43[:, :, j], e3[:, :, j], rs_bc[:, j:j + 1], None,
                    mybir.AluOpType.mult,
                )
            w_sb = smp.tile([P, T], F32, tag="w_sb")
            nc.vector.tensor_reduce(w_sb[:, :], w43, axis=mybir.AxisListType.X,
                                    op=mybir.AluOpType.add)

            # ---------------- top-k threshold ----------------
            t8 = smp.tile([P, 8], F32, tag="t8")
            nc.vector.max(t8[:, :], w_sb[:, :])
            t8_dr = drp.tile([P, 8], F32, tag="t8_dr")
            nc.sync.dma_start(out=t8_dr[:, :], in_=t8[:, :])
            cand = smp.tile([1, P * 8], F32, tag="cand")
            nc.sync.dma_start(out=cand[:, :],
                              in_=t8_dr[:, :].rearrange("p e -> (p e)"))
            c16 = smp.tile([1, 16], F32, tag="c16")
            nc.vector.max(c16[:, 0:8], cand[:, :])
            cand2 = smp.tile([1, P * 8], F32, tag="cand2")
            nc.vector.match_replace(cand2[:, :], c16[:, 0:8], cand[:, :],
                                    -1e30)
            nc.vector.max(c16[:, 8:16], cand2[:, :])

            # R2: AllGather per-shard top-16
            cc2_in = drp.tile([1, 16], F32, tag="cc2_in")
            cc2_out = drp.tile([1, 16 * cores], F32, tag="cc2_out")
            nc.sync.dma_start(out=cc2_in[:, :], in_=c16[:, :])
            nc.gpsimd.collective_compute(
                "AllGather", mybir.AluOpType.bypass, replica_groups=rg,
                ins=[cc2_in[:, :].opt()], outs=[cc2_out[:, :].opt()],
            )
            g_sb = smp.tile([1, 16 * cores], F32, tag="g_sb")
            nc.sync.dma_start(out=g_sb[:, :], in_=cc2_out[:, :])
            g8a = smp.tile([1, 8], F32, tag="g8a")
            nc.vector.max(g8a[:, :], g_sb[:, :])
            g_sb2 = smp.tile([1, 16 * cores], F32, tag="g_sb2")
            nc.vector.match_replace(g_sb2[:, :], g8a[:, :], g_sb[:, :], -1e30)
            g8b = smp.tile([1, 8], F32, tag="g8b")
            nc.vector.max(g8b[:, :], g_sb2[:, :])
            # threshold = global rank-(topk) value
            assert 8 < topk <= 16
            thr = g8b[:, topk - 9: topk - 8]
            thr_bc = smp.tile([P, 1], F32, tag="thr_bc")
            nc.gpsimd.partition_broadcast(thr_bc[:, :], thr)

            # ---------------- mask, second softmax numerators ----------------
            wm = smp.tile([P, T], F32, tag="wm")
            nc.vector.scalar_tensor_tensor(
                wm[:, :], w_sb[:, :], thr_bc[:, 0:1], w_sb[:, :],
                op0=mybir.AluOpType.is_lt, op1=mybir.AluOpType.mult,
            )
            u1 = smp.tile([P, T], F32, tag="u1")
            dsum = smp.tile([P, 1], F32, tag="dsum")
            nc.scalar.activation(u1[:, :], wm[:, :],
                                 mybir.ActivationFunctionType.Exp,
                                 accum_out=dsum[:, :])
            up = smp.tile([P, T], F32, tag="up")      # u' = exp(wm) - 1
            nc.vector.tensor_scalar_add(up[:, :], u1[:, :], -1.0)
            up_bf = smp.tile([P, T], BF16, tag="up_bf")
            nc.vector.tensor_copy(up_bf[:, :], up[:, :])

            ps_d = psp.tile([1, 1], F32, tag="ps_d")
            nc.tensor.matmul(ps_d[0:1, :], lhsT=ones_f32[:, :],
                             rhs=dsum[:, :], start=True, stop=True)

            # pass B: bag1 = sum_n u'[n] * f[n, :]
            ps_b1 = psp.tile([1, d], F32, tag="ps_b1")
            for t in range(T):
                for h0 in range(0, d, 512):
                    h1 = min(h0 + 512, d)
                    nc.tensor.matmul(
                        ps_b1[0:1, h0:h1],
                        lhsT=up_bf[:, t:t + 1],
                        rhs=fb[t][:, h0:h1],
                        start=(t == 0), stop=(t == T - 1),
                        skip_group_check=True,
                    )

            # R3: AllReduce [bag0+bag1 | denom]
            bagd = smp.tile([1, d + 1], F32, tag="bagd")
            nc.scalar.copy(bagd[:, 0:d], ps_b0[0:1, :])
            nc.vector.tensor_tensor(bagd[:, 0:d], bagd[:, 0:d], ps_b1[0:1, :],
                                    mybir.AluOpType.add)
            nc.scalar.copy(bagd[:, d:d + 1], ps_d[0:1, :])
            cc3_in = drp.tile([1, d + 1], F32, tag="cc3_in")
            cc3_out = drp.tile([1, d + 1], F32, tag="cc3_out")
            nc.sync.dma_start(out=cc3_in[:, :], in_=bagd[:, :])
            nc.gpsimd.collective_compute(
                "AllReduce", mybir.AluOpType.add, replica_groups=rg,
                ins=[cc3_in[:, :].opt()], outs=[cc3_out[:, :].opt()],
            )
            gb = smp.tile([1, d + 1], F32, tag="gb")
            nc.sync.dma_start(out=gb[:, :], in_=cc3_out[:, :])

            # ---------------- finals ----------------
            den = smp.tile([1, 1], F32, tag="den")
            nc.vector.tensor_scalar_add(den[:, :], gb[:, d:d + 1],
                                        float(-NPAD_L * cores))
            rden = smp.tile([1, 1], F32, tag="rden")
            nc.vector.reciprocal(rden[:, :], den[:, :])
            bag_o = smp.tile([1, d], F32, tag="bag_o")
            nc.vector.tensor_scalar(bag_o[:, :], gb[:, 0:d], rden[:, 0:1],
                                    None, mybir.AluOpType.mult)
            nc.sync.dma_start(out=out_bag, in_=bag_o[:, :])

            rden_bc = smp.tile([P, 1], F32, tag="rden_bc")
            nc.gpsimd.partition_broadcast(rden_bc[:, :], rden[:, :])
            w2 = smp.tile([P, T], F32, tag="w2")
            nc.vector.tensor_scalar(w2[:, :], u1[:, :], rden_bc[:, 0:1], None,
                                    mybir.AluOpType.mult)
            ps_w2t = psp.tile([T, P], F32, tag="ps_w2t")
            nc.tensor.matmul(ps_w2t[:, :], lhsT=w2[:, :], rhs=ident[:, :],
                             is_transpose=True, start=True, stop=True)
            w2t = smp.tile([T, P], F32, tag="w2t")
            nc.vector.tensor_copy(w2t[:, :], ps_w2t[:, :])
            nc.sync.dma_start(
                out=out_w[0:(T - 1) * P].rearrange("(t p) -> t p", p=P),
                in_=w2t[0:T - 1, :],
            )
            nc.sync.dma_start(
                out=out_w[(T - 1) * P: ns],
                in_=w2t[T - 1: T, 0:PROWS],
            )

    nc.compile()
    return nc


_NC_CACHE = {}


def _get_nc():
    if "nc" not in _NC_CACHE:
        _NC_CACHE["nc"] = build_nc()
    return _NC_CACHE["nc"]


def make_in_maps(features, W):
    wt = np.ascontiguousarray(W.T).astype(ml_dtypes.bfloat16)
    return [
        {"features": np.ascontiguousarray(features[c * NS:(c + 1) * NS]),
         "wt": wt}
        for c in range(CORES)
    ]


def kernel(features, W, b=None, **_ignored):
    features = np.asarray(features, dtype=np.float32)
    W = np.asarray(W, dtype=np.float32)
    nc = _get_nc()
    res = run_bass_kernel_spmd(nc, make_in_maps(features, W),
                               core_ids=list(range(CORES)))
    results = res.results
    bag = np.asarray(results[0]["out_bag"], dtype=np.float32)
    w = np.concatenate(
        [np.asarray(results[c]["out_w"], dtype=np.float32)
         for c in range(CORES)]
    )
    return bag, w


if __name__ == "__main__":
    nc = build_nc()
    print("build+compile OK;",
          sum(len(bb.instructions) for bb in nc.main_func.blocks),
          "instructions")


# revision 7
# speedup vs baseline: 1.0050x; 1.0050x over previous
"""ACMIL top-k masking kernel for 8 TRN2 NeuronCores.

Reference computation (N=50000, D=1024, BRANCHES=4, TOP_K=10):
    scores = features @ W.T + b          # [N, 4]   (b cancels in softmax)
    weights = softmax(scores, axis=0)    # over instances
    w = weights.mean(axis=1)             # [N]
    w[top_k(w, 10)] = 0
    w = softmax(w, axis=0)
    bag = w @ features                   # [D]
    returns (bag, w)

Distribution: shard instances (dim 0) across 8 cores (6250 rows each).
AllReduce the per-branch softmax normalizer, global top-k via per-shard
top-16 + AllGather + local rank-10 threshold, AllReduce the final pooled
bag + second-softmax denominator.
"""

import sys

for _p in ("/opt/trn_rl_repo",):
    if _p not in sys.path:
        sys.path.insert(0, _p)

import numpy as np
import ml_dtypes

import concourse.bass as bass
import concourse.bacc as bacc
import concourse.mybir as mybir
import concourse.tile as tile
from concourse.bass_utils import run_bass_kernel_spmd

F32 = mybir.dt.float32
BF16 = mybir.dt.bfloat16
I32 = mybir.dt.int32

N, D, BR, TOPK, CORES = 50000, 1024, 4, 10, 8
NS = N // CORES  # 6250 rows per core


def build_nc(ns=NS, d=D, br=BR, cores=CORES, use_dma_cast=True):
    """Build the per-core Bass graph (SPMD: same graph on all cores)."""
    P = 128
    T = (ns + P - 1) // P          # n-tiles per shard
    PROWS = ns - (T - 1) * P       # real rows in last tile
    NPAD_L = T * P - ns            # pad rows per core
    DC = d // P                    # d-chunks
    topk = TOPK

    nc = bacc.Bacc("TRN2", target_bir_lowering=False, debug=False,
                   num_devices=cores)

    feats = nc.dram_tensor("features", [ns, d], F32, kind="ExternalInput").ap()
    # host passes W.T pre-cast to bf16 (4KB, layout prep only)
    wt_in = nc.dram_tensor("wt", [d, br], BF16, kind="ExternalInput").ap()
    out_w = nc.dram_tensor("out_w", [ns], F32, kind="ExternalOutput").ap()
    out_bag = nc.dram_tensor("out_bag", [d], F32, kind="ExternalOutput").ap()

    rg = [list(range(cores))]

    with tile.TileContext(nc) as tc:
        with (
            tc.tile_pool(name="fb", bufs=1) as fbp,       # resident bf16 tiles
            tc.tile_pool(name="ft", bufs=12) as ftp,       # fT stream
            tc.tile_pool(name="sm", bufs=1) as smp,       # small persistents
            tc.tile_pool(name="ps", bufs=1, space="PSUM") as psp,
            tc.tile_pool(name="dr", bufs=1, space="DRAM") as drp,
        ):
            # ---------------- setup ----------------
            wt_sb = smp.tile([P, DC * br], BF16, tag="wt_sb")
            nc.sync.dma_start(
                out=wt_sb[:, :].rearrange("p (c b) -> p c b", b=br),
                in_=wt_in.rearrange("(c p) b -> p c b", p=P),
            )
            ones_bf = smp.tile([P, 1], BF16, tag="ones_bf")
            nc.gpsimd.memset(ones_bf[:, :], 1.0)
            ones_f32 = smp.tile([P, 1], F32, tag="ones_f32")
            nc.gpsimd.memset(ones_f32[:, :], 1.0)

            # identity matrix for PE transpose of the output weights
            iota_j = smp.tile([P, P], I32, tag="iota_j")
            nc.gpsimd.iota(iota_j[:, :], pattern=[[1, P]], base=0,
                           channel_multiplier=0)
            iota_p = smp.tile([P, 1], I32, tag="iota_p")
            nc.gpsimd.iota(iota_p[:, :], pattern=[[0, 1]], base=0,
                           channel_multiplier=1)
            iota_jf = smp.tile([P, P], F32, tag="iota_jf")
            nc.vector.tensor_copy(iota_jf[:, :], iota_j[:, :])
            iota_pf = smp.tile([P, 1], F32, tag="iota_pf")
            nc.vector.tensor_copy(iota_pf[:, :], iota_p[:, :])
            ident = smp.tile([P, P], F32, tag="ident")
            nc.vector.tensor_scalar(ident[:, :], iota_jf[:, :],
                                    iota_pf[:, :], None,
                                    mybir.AluOpType.is_equal)

            # ---------------- phase A: load + scores + bag0 ----------------
            ps_sc = psp.tile([P, T * br], F32, tag="ps_sc")    # score accum
            ps_b0 = psp.tile([1, d], F32, tag="ps_b0")         # sum_n f[n,:]

            fb = []
            for t in range(T):
                fb_t = fbp.tile([P, d], BF16, tag=f"fb{t}")
                fb.append(fb_t)
                rows = PROWS if t == T - 1 else P
                if rows < P:
                    nc.gpsimd.memset(fb_t[:, :], 0.0)
                if use_dma_cast:
                    nc.gpsimd.dma_start(out=fb_t[0:rows, :],
                                        in_=feats[t * P: t * P + rows, :])
                else:
                    fs_t = ftp.tile([P, d], F32, tag="fs")
                    nc.sync.dma_start(out=fs_t[0:rows, :],
                                      in_=feats[t * P: t * P + rows, :])
                    nc.scalar.copy(fb_t[0:rows, :], fs_t[0:rows, :])

                ft_t = ftp.tile([P, d], BF16, tag="ft")
                nc.sync.dma_start_transpose(
                    out=ft_t[:, :].rearrange("p (c j) -> p c j", c=DC),
                    in_=fb_t[:, :],
                )
                for c in range(DC):
                    nc.tensor.matmul(
                        ps_sc[:, t * br:(t + 1) * br],
                        lhsT=ft_t[:, c * P:(c + 1) * P],
                        rhs=wt_sb[:, c * br:(c + 1) * br],
                        start=(c == 0), stop=(c == DC - 1),
                    )
                for h0 in range(0, d, 512):
                    h1 = min(h0 + 512, d)
                    nc.tensor.matmul(
                        ps_b0[0:1, h0:h1],
                        lhsT=ones_bf[:, :],
                        rhs=fb_t[:, h0:h1],
                        start=(t == 0), stop=(t == T - 1),
                        skip_group_check=True,
                    )

            # ---------------- e = exp(scores), branch sums ----------------
            e_sb = smp.tile([P, T * br], F32, tag="e_sb")
            nc.scalar.activation(e_sb[:, :], ps_sc[:, :],
                                 mybir.ActivationFunctionType.Exp)
            sp = smp.tile([P, br], F32, tag="sp")  # per-partition branch sums
            nc.vector.tensor_reduce(
                sp[:, :],
                e_sb[:, :].rearrange("p (t b) -> p b t", b=br),
                axis=mybir.AxisListType.X, op=mybir.AluOpType.add,
            )
            ps_s = psp.tile([1, br], F32, tag="ps_s")
            nc.tensor.matmul(ps_s[0:1, :], lhsT=ones_f32[:, :], rhs=sp[:, :],
                             start=True, stop=True)
            s_row = smp.tile([1, br], F32, tag="s_row")
            nc.scalar.copy(s_row[:, :], ps_s[0:1, :])

            # R1: AllReduce branch normalizers
            cc1_in = drp.tile([1, br], F32, tag="cc1_in")
            cc1_out = drp.tile([1, br], F32, tag="cc1_out")
            nc.sync.dma_start(out=cc1_in[:, :], in_=s_row[:, :])
            nc.gpsimd.collective_compute(
                "AllReduce", mybir.AluOpType.add, replica_groups=rg,
                ins=[cc1_in[:, :].opt()], outs=[cc1_out[:, :].opt()],
            )
            sg = smp.tile([1, br], F32, tag="sg")
            nc.sync.dma_start(out=sg[:, :], in_=cc1_out[:, :])
            if NPAD_L > 0:
                nc.vector.tensor_scalar_add(sg[:, :], sg[:, :],
                                            float(-NPAD_L * cores))

            # w[n] = sum_br e[n,br] / (4*S_br)   -> [128, T]
            rs = smp.tile([1, br], F32, tag="rs")
            nc.vector.reciprocal(rs[:, :], sg[:, :])
            nc.vector.tensor_scalar_mul(rs[:, :], rs[:, :], 1.0 / br)
            rs_bc = smp.tile([P, br], F32, tag="rs_bc")
            nc.gpsimd.partition_broadcast(rs_bc[:, :], rs[:, :])
            w4 = smp.tile([P, T * br], F32, tag="w4")
            e3 = e_sb[:, :].rearrange("p (t b) -> p t b", b=br)
            w43 = w4[:, :].rearrange("p (t b) -> p t b", b=br)
            for j in range(br):
                nc.vector.tensor_scalar(
                    w43[:, :, j], e3[:, :, j], rs_bc[:, j:j + 1], None,
                    mybir.AluOpType.mult,
                )
            w_sb = smp.tile([P, T], F32, tag="w_sb")
            nc.vector.tensor_reduce(w_sb[:, :], w43, axis=mybir.AxisListType.X,
                                    op=mybir.AluOpType.add)

            # ---------------- top-k threshold ----------------
            t8 = smp.tile([P, 8], F32, tag="t8")
            nc.vector.max(t8[:, :], w_sb[:, :])
            t8_dr = drp.tile([P, 8], F32, tag="t8_dr")
            nc.sync.dma_start(out=t8_dr[:, :], in_=t8[:, :])
            cand = smp.tile([1, P * 8], F32, tag="cand")
            nc.sync.dma_start(out=cand[:, :],
                              in_=t8_dr[:, :].rearrange("p e -> (p e)"))
            c16 = smp.tile([1, 16], F32, tag="c16")
            nc.vector.max(c16[:, 0:8], cand[:, :])
            cand2 = smp.tile([1, P * 8], F32, tag="cand2")
            nc.vector.match_replace(cand2[:, :], c16[:, 0:8], cand[:, :],
                                    -1e30)
            nc.vector.max(c16[:, 8:16], cand2[:, :])

            # R2: AllGather per-shard top-16
            cc2_in = drp.tile([1, 16], F32, tag="cc2_in")
            cc2_out = drp.tile([1, 16 * cores], F32, tag="cc2_out")
            nc.sync.dma_start(out=cc2_in[:, :], in_=c16[:, :])
            nc.gpsimd.collective_compute(
                "AllGather", mybir.AluOpType.bypass, replica_groups=rg,
                ins=[cc2_in[:, :].opt()], outs=[cc2_out[:, :].opt()],
            )
            g_sb = smp.tile([1, 16 * cores], F32, tag="g_sb")
            nc.sync.dma_start(out=g_sb[:, :], in_=cc2_out[:, :])
            g8a = smp.tile([1, 8], F32, tag="g8a")
            nc.vector.max(g8a[:, :], g_sb[:, :])
            g_sb2 = smp.tile([1, 16 * cores], F32, tag="g_sb2")
            nc.vector.match_replace(g_sb2[:, :], g8a[:, :], g_sb[:, :], -1e30)
            g8b = smp.tile([1, 8], F32, tag="g8b")
            nc.vector.max(g8b[:, :], g_sb2[:, :])
            # threshold = global rank-(topk) value
            assert 8 < topk <= 16
            thr = g8b[:, topk - 9: topk - 8]
            thr_bc = smp.tile([P, 1], F32, tag="thr_bc")
            nc.gpsimd.partition_broadcast(thr_bc[:, :], thr)

            # ---------------- mask, second softmax numerators ----------------
            wm = smp.tile([P, T], F32, tag="wm")
            nc.vector.scalar_tensor_tensor(
                wm[:, :], w_sb[:, :], thr_bc[:, 0:1], w_sb[:, :],
                op0=mybir.AluOpType.is_lt, op1=mybir.AluOpType.mult,
            )
            u1 = smp.tile([P, T], F32, tag="u1")
            dsum = smp.tile([P, 1], F32, tag="dsum")
            nc.scalar.activation(u1[:, :], wm[:, :],
                                 mybir.ActivationFunctionType.Exp,
                                 accum_out=dsum[:, :])
            up = smp.tile([P, T], F32, tag="up")      # u' = exp(wm) - 1
            nc.vector.tensor_scalar_add(up[:, :], u1[:, :], -1.0)
            up_bf = smp.tile([P, T], BF16, tag="up_bf")
            nc.vector.tensor_copy(up_bf[:, :], up[:, :])

            ps_d = psp.tile([1, 1], F32, tag="ps_d")
            nc.tensor.matmul(ps_d[0:1, :], lhsT=ones_f32[:, :],
                             rhs=dsum[:, :], start=True, stop=True)

            # pass B: bag1 = sum_n u'[n] * f[n, :]
            ps_b1 = psp.tile([1, d], F32, tag="ps_b1")
            for t in range(T):
                for h0 in range(0, d, 512):
                    h1 = min(h0 + 512, d)
                    nc.tensor.matmul(
                        ps_b1[0:1, h0:h1],
                        lhsT=up_bf[:, t:t + 1],
                        rhs=fb[t][:, h0:h1],
                        start=(t == 0), stop=(t == T - 1),
                        skip_group_check=True,
                    )

            # R3: AllReduce [bag0+bag1 | denom]
            bagd = smp.tile([1, d + 1], F32, tag="bagd")
            nc.scalar.copy(bagd[:, 0:d], ps_b0[0:1, :])
            nc.vector.tensor_tensor(bagd[:, 0:d], bagd[:, 0:d], ps_b1[0:1, :],
                                    mybir.AluOpType.add)
            nc.scalar.copy(bagd[:, d:d + 1], ps_d[0:1, :])
            cc3_in = drp.tile([1, d + 1], F32, tag="cc3_in")
            cc3_out = drp.tile([1, d + 1], F32, tag="cc3_out")
            nc.sync.dma_start(out=cc3_in[:, :], in_=bagd[:, :])
            nc.gpsimd.collective_compute(
                "AllReduce", mybir.AluOpType.add, replica_groups=rg,
                ins=[cc3_in[:, :].opt()], outs=[cc3_out[:, :].opt()],
            )
            gb = smp.tile([1, d + 1], F32, tag="gb")
            nc.sync.dma_start(out=gb[:, :], in_=cc3_out[:, :])

            # ---------------- finals ----------------
            den = smp.tile([1, 1], F32, tag="den")
            nc.vector.tensor_scalar_add(den[:, :], gb[:, d:d + 1],
                                        float(-NPAD_L * cores))
            rden = smp.tile([1, 1], F32, tag="rden")
            nc.vector.reciprocal(rden[:, :], den[:, :])
            bag_o = smp.tile([1, d], F32, tag="bag_o")
            nc.vector.tensor_scalar(bag_o[:, :], gb[:, 0:d], rden[:, 0:1],
                                    None, mybir.AluOpType.mult)
            nc.sync.dma_start(out=out_bag, in_=bag_o[:, :])

            rden_bc = smp.tile([P, 1], F32, tag="rden_bc")
            nc.gpsimd.partition_broadcast(rden_bc[:, :], rden[:, :])
            w2 = smp.tile([P, T], F32, tag="w2")
            nc.vector.tensor_scalar(w2[:, :], u1[:, :], rden_bc[:, 0:1], None,
                                    mybir.AluOpType.mult)
            ps_w2t = psp.tile([T, P], F32, tag="ps_w2t")
            nc.tensor.matmul(ps_w2t[:, :], lhsT=w2[:, :], rhs=ident[:, :],
                             is_transpose=True, start=True, stop=True)
            w2t = smp.tile([T, P], F32, tag="w2t")
            nc.vector.tensor_copy(w2t[:, :], ps_w2t[:, :])
            nc.sync.dma_start(
                out=out_w[0:(T - 1) * P].rearrange("(t p) -> t p", p=P),
                in_=w2t[0:T - 1, :],
            )
            nc.sync.dma_start(
                out=out_w[(T - 1) * P: ns],
                in_=w2t[T - 1: T, 0:PROWS],
            )

    nc.compile()
    return nc


_NC_CACHE = {}


def _get_nc():
    if "nc" not in _NC_CACHE:
        _NC_CACHE["nc"] = build_nc()
    return _NC_CACHE["nc"]


def make_in_maps(features, W):
    wt = np.ascontiguousarray(W.T).astype(ml_dtypes.bfloat16)
    return [
        {"features": np.ascontiguousarray(features[c * NS:(c + 1) * NS]),
         "wt": wt}
        for c in range(CORES)
    ]


def kernel(features, W, b=None, **_ignored):
    features = np.asarray(features, dtype=np.float32)
    W = np.asarray(W, dtype=np.float32)
    nc = _get_nc()
    res = run_bass_kernel_spmd(nc, make_in_maps(features, W),
                               core_ids=list(range(CORES)))
    results = res.results
    bag = np.asarray(results[0]["out_bag"], dtype=np.float32)
    w = np.concatenate(
        [np.asarray(results[c]["out_w"], dtype=np.float32)
         for c in range(CORES)]
    )
    return bag, w


if __name__ == "__main__":
    nc = build_nc()
    print("build+compile OK;",
          sum(len(bb.instructions) for bb in nc.main_func.blocks),
          "instructions")


# revision 9
# speedup vs baseline: 1.9644x; 1.9546x over previous
"""ACMIL top-k masking kernel for 8 TRN2 NeuronCores.

Reference computation (N=50000, D=1024, BRANCHES=4, TOP_K=10):
    scores = features @ W.T + b          # [N, 4]   (b cancels in softmax)
    weights = softmax(scores, axis=0)    # over instances
    w = weights.mean(axis=1)             # [N]
    w[top_k(w, 10)] = 0
    w = softmax(w, axis=0)
    bag = w @ features                   # [D]
    returns (bag, w)

Distribution: shard instances (dim 0) across 8 cores (6250 rows each).
AllReduce the per-branch softmax normalizer, global top-k via per-shard
top-16 + AllGather + local rank-10 threshold, AllReduce the final pooled
bag + second-softmax denominator.
"""

import sys

for _p in ("/opt/trn_rl_repo",):
    if _p not in sys.path:
        sys.path.insert(0, _p)

import numpy as np
import ml_dtypes

import concourse.bass as bass
import concourse.bacc as bacc
import concourse.mybir as mybir
import concourse.tile as tile
from concourse.bass_utils import run_bass_kernel_spmd

F32 = mybir.dt.float32
BF16 = mybir.dt.bfloat16
I32 = mybir.dt.int32

N, D, BR, TOPK, CORES = 50000, 1024, 4, 10, 8
NS = N // CORES  # 6250 rows per core


def build_nc(ns=NS, d=D, br=BR, cores=CORES, use_dma_cast=True):
    """Build the per-core Bass graph (SPMD: same graph on all cores)."""
    P = 128
    T = (ns + P - 1) // P          # n-tiles per shard
    PROWS = ns - (T - 1) * P       # real rows in last tile
    NPAD_L = T * P - ns            # pad rows per core
    DC = d // P                    # d-chunks
    topk = TOPK

    nc = bacc.Bacc("TRN2", target_bir_lowering=False, debug=False,
                   num_devices=cores)

    feats = nc.dram_tensor("features", [ns, d], F32, kind="ExternalInput").ap()
    # host passes W.T pre-cast to bf16 (4KB, layout prep only)
    wt_in = nc.dram_tensor("wt", [d, br], BF16, kind="ExternalInput").ap()
    out_w = nc.dram_tensor("out_w", [ns], F32, kind="ExternalOutput").ap()
    out_bag = nc.dram_tensor("out_bag", [d], F32, kind="ExternalOutput").ap()

    rg = [list(range(cores))]

    with tile.TileContext(nc) as tc:
        with (
            tc.tile_pool(name="fb", bufs=1) as fbp,       # resident bf16 tiles
            tc.tile_pool(name="ft", bufs=12) as ftp,       # fT stream
            tc.tile_pool(name="sm", bufs=1) as smp,       # small persistents
            tc.tile_pool(name="ps", bufs=1, space="PSUM") as psp,
            tc.tile_pool(name="dr", bufs=1, space="DRAM") as drp,
        ):
            # ---------------- setup ----------------
            wt_sb = smp.tile([P, DC * br], BF16, tag="wt_sb")
            nc.sync.dma_start(
                out=wt_sb[:, :].rearrange("p (c b) -> p c b", b=br),
                in_=wt_in.rearrange("(c p) b -> p c b", p=P),
            )
            ones_bf = smp.tile([P, 1], BF16, tag="ones_bf")
            nc.gpsimd.memset(ones_bf[:, :], 1.0)
            ones_f32 = smp.tile([P, 1], F32, tag="ones_f32")
            nc.gpsimd.memset(ones_f32[:, :], 1.0)

            # identity matrix for PE transpose of the output weights
            iota_j = smp.tile([P, P], I32, tag="iota_j")
            nc.gpsimd.iota(iota_j[:, :], pattern=[[1, P]], base=0,
                           channel_multiplier=0)
            iota_p = smp.tile([P, 1], I32, tag="iota_p")
            nc.gpsimd.iota(iota_p[:, :], pattern=[[0, 1]], base=0,
                           channel_multiplier=1)
            iota_jf = smp.tile([P, P], F32, tag="iota_jf")
            nc.vector.tensor_copy(iota_jf[:, :], iota_j[:, :])
            iota_pf = smp.tile([P, 1], F32, tag="iota_pf")
            nc.vector.tensor_copy(iota_pf[:, :], iota_p[:, :])
            ident = smp.tile([P, P], F32, tag="ident")
            nc.vector.tensor_scalar(ident[:, :], iota_jf[:, :],
                                    iota_pf[:, :], None,
                                    mybir.AluOpType.is_equal)
            ident_bf = smp.tile([P, P], BF16, tag="ident_bf")
            nc.vector.tensor_copy(ident_bf[:, :], ident[:, :])

            # ---------------- phase A: load + scores + bag0 ----------------
            ps_sc = psp.tile([P, T * br], F32, tag="ps_sc")    # score accum
            # bag psum accumulates BOTH sum_n f (phase A) and
            # sum_n u'*f (pass B) -- one accumulation group.
            ps_bag = psp.tile([1, d], F32, tag="ps_bag")

            fb = []
            for t in range(T):
                fb_t = fbp.tile([P, d], BF16, tag=f"fb{t}")
                fb.append(fb_t)
                rows = PROWS if t == T - 1 else P
                if rows < P:
                    nc.gpsimd.memset(fb_t[:, :], 0.0)
                if use_dma_cast:
                    nc.gpsimd.dma_start(out=fb_t[0:rows, :],
                                        in_=feats[t * P: t * P + rows, :])
                else:
                    fs_t = ftp.tile([P, d], F32, tag="fs")
                    nc.sync.dma_start(out=fs_t[0:rows, :],
                                      in_=feats[t * P: t * P + rows, :])
                    nc.scalar.copy(fb_t[0:rows, :], fs_t[0:rows, :])

                # transpose on PE via identity matmul, one PSUM bank per tile
                ptr_t = psp.tile([P, d], BF16, tag="ptr", bufs=2)
                for c in range(DC):
                    nc.tensor.transpose(ptr_t[:, c * P:(c + 1) * P],
                                        fb_t[:, c * P:(c + 1) * P],
                                        ident_bf[:, :])
                ft_t = ftp.tile([P, d], BF16, tag="ft")
                if t % 2 == 0:
                    nc.vector.tensor_copy(ft_t[:, :], ptr_t[:, :])
                else:
                    nc.scalar.copy(ft_t[:, :], ptr_t[:, :])
                for c in range(DC):
                    nc.tensor.matmul(
                        ps_sc[:, t * br:(t + 1) * br],
                        lhsT=ft_t[:, c * P:(c + 1) * P],
                        rhs=wt_sb[:, c * br:(c + 1) * br],
                        start=(c == 0), stop=(c == DC - 1),
                    )
                for h0 in range(0, d, 512):
                    h1 = min(h0 + 512, d)
                    nc.tensor.matmul(
                        ps_bag[0:1, h0:h1],
                        lhsT=ones_bf[:, :],
                        rhs=fb_t[:, h0:h1],
                        start=(t == 0), stop=False,
                        skip_group_check=True,
                    )

            # ---------------- e = exp(scores), branch sums ----------------
            e_sb = smp.tile([P, T * br], F32, tag="e_sb")
            nc.scalar.activation(e_sb[:, :], ps_sc[:, :],
                                 mybir.ActivationFunctionType.Exp)
            sp = smp.tile([P, br], F32, tag="sp")  # per-partition branch sums
            nc.vector.tensor_reduce(
                sp[:, :],
                e_sb[:, :].rearrange("p (t b) -> p b t", b=br),
                axis=mybir.AxisListType.X, op=mybir.AluOpType.add,
            )
            ps_s = psp.tile([1, br], F32, tag="ps_s")
            nc.tensor.matmul(ps_s[0:1, :], lhsT=ones_f32[:, :], rhs=sp[:, :],
                             start=True, stop=True)
            s_row = smp.tile([1, br], F32, tag="s_row")
            nc.scalar.copy(s_row[:, :], ps_s[0:1, :])

            # R1: AllReduce branch normalizers
            cc1_in = drp.tile([1, br], F32, tag="cc1_in")
            cc1_out = drp.tile([1, br], F32, tag="cc1_out")
            nc.sync.dma_start(out=cc1_in[:, :], in_=s_row[:, :])
            nc.gpsimd.collective_compute(
                "AllReduce", mybir.AluOpType.add, replica_groups=rg,
                ins=[cc1_in[:, :].opt()], outs=[cc1_out[:, :].opt()],
            )
            sg = smp.tile([1, br], F32, tag="sg")
            nc.sync.dma_start(out=sg[:, :], in_=cc1_out[:, :])
            if NPAD_L > 0:
                nc.vector.tensor_scalar_add(sg[:, :], sg[:, :],
                                            float(-NPAD_L * cores))

            # w[n] = sum_br e[n,br] / (4*S_br)   -> [128, T]
            rs = smp.tile([1, br], F32, tag="rs")
            nc.vector.reciprocal(rs[:, :], sg[:, :])
            nc.vector.tensor_scalar_mul(rs[:, :], rs[:, :], 1.0 / br)
            rs_bc = smp.tile([P, br], F32, tag="rs_bc")
            nc.gpsimd.partition_broadcast(rs_bc[:, :], rs[:, :])
            w4 = smp.tile([P, T * br], F32, tag="w4")
            e3 = e_sb[:, :].rearrange("p (t b) -> p t b", b=br)
            w43 = w4[:, :].rearrange("p (t b) -> p t b", b=br)
            for j in range(br):
                nc.vector.tensor_scalar(
                    w43[:, :, j], e3[:, :, j], rs_bc[:, j:j + 1], None,
                    mybir.AluOpType.mult,
                )
            w_sb = smp.tile([P, T], F32, tag="w_sb")
            nc.vector.tensor_reduce(w_sb[:, :], w43, axis=mybir.AxisListType.X,
                                    op=mybir.AluOpType.add)

            # ---------------- top-k threshold ----------------
            t8 = smp.tile([P, 8], F32, tag="t8")
            nc.vector.max(t8[:, :], w_sb[:, :])
            t8_dr = drp.tile([P, 8], F32, tag="t8_dr")
            nc.sync.dma_start(out=t8_dr[:, :], in_=t8[:, :])
            cand = smp.tile([1, P * 8], F32, tag="cand")
            nc.sync.dma_start(out=cand[:, :],
                              in_=t8_dr[:, :].rearrange("p e -> (p e)"))
            c16 = smp.tile([1, 16], F32, tag="c16")
            nc.vector.max(c16[:, 0:8], cand[:, :])
            cand2 = smp.tile([1, P * 8], F32, tag="cand2")
            nc.vector.match_replace(cand2[:, :], c16[:, 0:8], cand[:, :],
                                    -1e30)
            nc.vector.max(c16[:, 8:16], cand2[:, :])

            # R2: AllGather per-shard top-16
            cc2_in = drp.tile([1, 16], F32, tag="cc2_in")
            cc2_out = drp.tile([1, 16 * cores], F32, tag="cc2_out")
            nc.sync.dma_start(out=cc2_in[:, :], in_=c16[:, :])
            nc.gpsimd.collective_compute(
                "AllGather", mybir.AluOpType.bypass, replica_groups=rg,
                ins=[cc2_in[:, :].opt()], outs=[cc2_out[:, :].opt()],
            )
            g_sb = smp.tile([1, 16 * cores], F32, tag="g_sb")
            nc.sync.dma_start(out=g_sb[:, :], in_=cc2_out[:, :])
            g8a = smp.tile([1, 8], F32, tag="g8a")
            nc.vector.max(g8a[:, :], g_sb[:, :])
            g_sb2 = smp.tile([1, 16 * cores], F32, tag="g_sb2")
            nc.vector.match_replace(g_sb2[:, :], g8a[:, :], g_sb[:, :], -1e30)
            g8b = smp.tile([1, 8], F32, tag="g8b")
            nc.vector.max(g8b[:, :], g_sb2[:, :])
            # threshold = global rank-(topk) value
            assert 8 < topk <= 16
            thr = g8b[:, topk - 9: topk - 8]
            thr_bc = smp.tile([P, 1], F32, tag="thr_bc")
            nc.gpsimd.partition_broadcast(thr_bc[:, :], thr)

            # ---------------- mask, second softmax numerators ----------------
            wm = smp.tile([P, T], F32, tag="wm")
            nc.vector.scalar_tensor_tensor(
                wm[:, :], w_sb[:, :], thr_bc[:, 0:1], w_sb[:, :],
                op0=mybir.AluOpType.is_lt, op1=mybir.AluOpType.mult,
            )
            u1 = smp.tile([P, T], F32, tag="u1")
            dsum = smp.tile([P, 1], F32, tag="dsum")
            nc.scalar.activation(u1[:, :], wm[:, :],
                                 mybir.ActivationFunctionType.Exp,
                                 accum_out=dsum[:, :])
            up = smp.tile([P, T], F32, tag="up")      # u' = exp(wm) - 1
            nc.vector.tensor_scalar_add(up[:, :], u1[:, :], -1.0)
            up_bf = smp.tile([P, T], BF16, tag="up_bf")
            nc.vector.tensor_copy(up_bf[:, :], up[:, :])

            ps_d = psp.tile([1, 1], F32, tag="ps_d")
            nc.tensor.matmul(ps_d[0:1, :], lhsT=ones_f32[:, :],
                             rhs=dsum[:, :], start=True, stop=True)

            # pass B: accumulate sum_n u'[n] * f[n, :] into the same psum
            for t in range(T):
                for h0 in range(0, d, 512):
                    h1 = min(h0 + 512, d)
                    nc.tensor.matmul(
                        ps_bag[0:1, h0:h1],
                        lhsT=up_bf[:, t:t + 1],
                        rhs=fb[t][:, h0:h1],
                        start=False, stop=(t == T - 1),
                        skip_group_check=True,
                    )

            # R3: AllReduce [bag0+bag1 | denom]
            bagd = smp.tile([1, d + 1], F32, tag="bagd")
            nc.scalar.copy(bagd[:, 0:d], ps_bag[0:1, :])
            nc.scalar.copy(bagd[:, d:d + 1], ps_d[0:1, :])
            cc3_in = drp.tile([1, d + 1], F32, tag="cc3_in")
            cc3_out = drp.tile([1, d + 1], F32, tag="cc3_out")
            nc.sync.dma_start(out=cc3_in[:, :], in_=bagd[:, :])
            nc.gpsimd.collective_compute(
                "AllReduce", mybir.AluOpType.add, replica_groups=rg,
                ins=[cc3_in[:, :].opt()], outs=[cc3_out[:, :].opt()],
            )
            gb = smp.tile([1, d + 1], F32, tag="gb")
            nc.sync.dma_start(out=gb[:, :], in_=cc3_out[:, :])

            # ---------------- finals ----------------
            den = smp.tile([1, 1], F32, tag="den")
            nc.vector.tensor_scalar_add(den[:, :], gb[:, d:d + 1],
                                        float(-NPAD_L * cores))
            rden = smp.tile([1, 1], F32, tag="rden")
            nc.vector.reciprocal(rden[:, :], den[:, :])
            bag_o = smp.tile([1, d], F32, tag="bag_o")
            nc.vector.tensor_scalar(bag_o[:, :], gb[:, 0:d], rden[:, 0:1],
                                    None, mybir.AluOpType.mult)
            nc.sync.dma_start(out=out_bag, in_=bag_o[:, :])

            rden_bc = smp.tile([P, 1], F32, tag="rden_bc")
            nc.gpsimd.partition_broadcast(rden_bc[:, :], rden[:, :])
            w2 = smp.tile([P, T], F32, tag="w2")
            nc.vector.tensor_scalar(w2[:, :], u1[:, :], rden_bc[:, 0:1], None,
                                    mybir.AluOpType.mult)
            ps_w2t = psp.tile([T, P], F32, tag="ps_w2t")
            nc.tensor.matmul(ps_w2t[:, :], lhsT=w2[:, :], rhs=ident[:, :],
                             is_transpose=True, start=True, stop=True)
            w2t = smp.tile([T, P], F32, tag="w2t")
            nc.vector.tensor_copy(w2t[:, :], ps_w2t[:, :])
            nc.sync.dma_start(
                out=out_w[0:(T - 1) * P].rearrange("(t p) -> t p", p=P),
                in_=w2t[0:T - 1, :],
            )
            nc.sync.dma_start(
                out=out_w[(T - 1) * P: ns],
                in_=w2t[T - 1: T, 0:PROWS],
            )

    nc.compile()
    return nc


_NC_CACHE = {}


def _get_nc():
    if "nc" not in _NC_CACHE:
        _NC_CACHE["nc"] = build_nc()
    return _NC_CACHE["nc"]


def make_in_maps(features, W):
    wt = np.ascontiguousarray(W.T).astype(ml_dtypes.bfloat16)
    return [
        {"features": np.ascontiguousarray(features[c * NS:(c + 1) * NS]),
         "wt": wt}
        for c in range(CORES)
    ]


def kernel(features, W, b=None, **_ignored):
    features = np.asarray(features, dtype=np.float32)
    W = np.asarray(W, dtype=np.float32)
    nc = _get_nc()
    res = run_bass_kernel_spmd(nc, make_in_maps(features, W),
                               core_ids=list(range(CORES)))
    results = res.results
    bag = np.asarray(results[0]["out_bag"], dtype=np.float32)
    w = np.concatenate(
        [np.asarray(results[c]["out_w"], dtype=np.float32)
         for c in range(CORES)]
    )
    return bag, w


if __name__ == "__main__":
    nc = build_nc()
    print("build+compile OK;",
          sum(len(bb.instructions) for bb in nc.main_func.blocks),
          "instructions")


# revision 10
# speedup vs baseline: 2.0824x; 1.0601x over previous
"""ACMIL top-k masking kernel for 8 TRN2 NeuronCores.

Reference computation (N=50000, D=1024, BRANCHES=4, TOP_K=10):
    scores = features @ W.T + b          # [N, 4]   (b cancels in softmax)
    weights = softmax(scores, axis=0)    # over instances
    w = weights.mean(axis=1)             # [N]
    w[top_k(w, 10)] = 0
    w = softmax(w, axis=0)
    bag = w @ features                   # [D]
    returns (bag, w)

Distribution: shard instances (dim 0) across 8 cores (6250 rows each).
AllReduce the per-branch softmax normalizer, global top-k via per-shard
top-16 + AllGather + local rank-10 threshold, AllReduce the final pooled
bag + second-softmax denominator.
"""

import sys

for _p in ("/opt/trn_rl_repo",):
    if _p not in sys.path:
        sys.path.insert(0, _p)

import numpy as np
import ml_dtypes

import concourse.bass as bass
import concourse.bacc as bacc
import concourse.mybir as mybir
import concourse.tile as tile
from concourse.bass_utils import run_bass_kernel_spmd

F32 = mybir.dt.float32
BF16 = mybir.dt.bfloat16
I32 = mybir.dt.int32

N, D, BR, TOPK, CORES = 50000, 1024, 4, 10, 8
NS = N // CORES  # 6250 rows per core


def build_nc(ns=NS, d=D, br=BR, cores=CORES, use_dma_cast=False):
    """Build the per-core Bass graph (SPMD: same graph on all cores)."""
    P = 128
    T = (ns + P - 1) // P          # n-tiles per shard
    PROWS = ns - (T - 1) * P       # real rows in last tile
    NPAD_L = T * P - ns            # pad rows per core
    DC = d // P                    # d-chunks
    topk = TOPK

    nc = bacc.Bacc("TRN2", target_bir_lowering=False, debug=False,
                   num_devices=cores)

    feats = nc.dram_tensor("features", [ns, d], F32, kind="ExternalInput").ap()
    # host passes W.T pre-cast to bf16 (4KB, layout prep only)
    wt_in = nc.dram_tensor("wt", [d, br], BF16, kind="ExternalInput").ap()
    out_w = nc.dram_tensor("out_w", [ns], F32, kind="ExternalOutput").ap()
    out_bag = nc.dram_tensor("out_bag", [d], F32, kind="ExternalOutput").ap()

    rg = [list(range(cores))]

    with tile.TileContext(nc) as tc:
        with (
            tc.tile_pool(name="fb", bufs=1) as fbp,       # resident bf16 tiles
            tc.tile_pool(name="ft", bufs=6) as ftp,       # fT stream
            tc.tile_pool(name="sm", bufs=1) as smp,       # small persistents
            tc.tile_pool(name="ps", bufs=1, space="PSUM") as psp,
            tc.tile_pool(name="dr", bufs=1, space="DRAM") as drp,
        ):
            # ---------------- setup ----------------
            wt_sb = smp.tile([P, DC * br], BF16, tag="wt_sb")
            nc.sync.dma_start(
                out=wt_sb[:, :].rearrange("p (c b) -> p c b", b=br),
                in_=wt_in.rearrange("(c p) b -> p c b", p=P),
            )
            ones_bf = smp.tile([P, 1], BF16, tag="ones_bf")
            nc.gpsimd.memset(ones_bf[:, :], 1.0)
            ones_f32 = smp.tile([P, 1], F32, tag="ones_f32")
            nc.gpsimd.memset(ones_f32[:, :], 1.0)

            # identity matrix for PE transpose of the output weights
            iota_j = smp.tile([P, P], I32, tag="iota_j")
            nc.gpsimd.iota(iota_j[:, :], pattern=[[1, P]], base=0,
                           channel_multiplier=0)
            iota_p = smp.tile([P, 1], I32, tag="iota_p")
            nc.gpsimd.iota(iota_p[:, :], pattern=[[0, 1]], base=0,
                           channel_multiplier=1)
            iota_jf = smp.tile([P, P], F32, tag="iota_jf")
            nc.vector.tensor_copy(iota_jf[:, :], iota_j[:, :])
            iota_pf = smp.tile([P, 1], F32, tag="iota_pf")
            nc.vector.tensor_copy(iota_pf[:, :], iota_p[:, :])
            ident = smp.tile([P, P], F32, tag="ident")
            nc.vector.tensor_scalar(ident[:, :], iota_jf[:, :],
                                    iota_pf[:, :], None,
                                    mybir.AluOpType.is_equal)
            ident_bf = smp.tile([P, P], BF16, tag="ident_bf")
            nc.vector.tensor_copy(ident_bf[:, :], ident[:, :])

            # ---------------- phase A: load + scores + bag0 ----------------
            ps_sc = psp.tile([P, T * br], F32, tag="ps_sc")    # score accum
            # bag psum accumulates BOTH sum_n f (phase A) and
            # sum_n u'*f (pass B) -- one accumulation group.
            ps_bag = psp.tile([1, d], F32, tag="ps_bag")

            fb = []
            for t in range(T):
                fb_t = fbp.tile([P, d], BF16, tag=f"fb{t}")
                fb.append(fb_t)
                rows = PROWS if t == T - 1 else P
                if rows < P:
                    nc.gpsimd.memset(fb_t[:, :], 0.0)
                if use_dma_cast:
                    nc.gpsimd.dma_start(out=fb_t[0:rows, :],
                                        in_=feats[t * P: t * P + rows, :])
                else:
                    fs_t = ftp.tile([P, d], F32, tag="fs")
                    nc.sync.dma_start(out=fs_t[0:rows, :],
                                      in_=feats[t * P: t * P + rows, :])
                    if t % 2 == 0:
                        nc.scalar.copy(fb_t[0:rows, :], fs_t[0:rows, :])
                    else:
                        nc.vector.tensor_copy(fb_t[0:rows, :],
                                              fs_t[0:rows, :])

                # transpose on PE via identity matmul, one PSUM bank per tile
                ptr_t = psp.tile([P, d], BF16, tag="ptr", bufs=2)
                for c in range(DC):
                    nc.tensor.transpose(ptr_t[:, c * P:(c + 1) * P],
                                        fb_t[:, c * P:(c + 1) * P],
                                        ident_bf[:, :])
                ft_t = ftp.tile([P, d], BF16, tag="ft")
                if t % 2 == 1:
                    nc.vector.tensor_copy(ft_t[:, :], ptr_t[:, :])
                else:
                    nc.scalar.copy(ft_t[:, :], ptr_t[:, :])
                for c in range(DC):
                    nc.tensor.matmul(
                        ps_sc[:, t * br:(t + 1) * br],
                        lhsT=ft_t[:, c * P:(c + 1) * P],
                        rhs=wt_sb[:, c * br:(c + 1) * br],
                        start=(c == 0), stop=(c == DC - 1),
                    )

            # ---------------- e = exp(scores), branch sums ----------------
            e_sb = smp.tile([P, T * br], F32, tag="e_sb")
            nc.scalar.activation(e_sb[:, :], ps_sc[:, :],
                                 mybir.ActivationFunctionType.Exp)
            sp = smp.tile([P, br], F32, tag="sp")  # per-partition branch sums
            nc.vector.tensor_reduce(
                sp[:, :],
                e_sb[:, :].rearrange("p (t b) -> p b t", b=br),
                axis=mybir.AxisListType.X, op=mybir.AluOpType.add,
            )
            ps_s = psp.tile([1, br], F32, tag="ps_s")
            nc.tensor.matmul(ps_s[0:1, :], lhsT=ones_f32[:, :], rhs=sp[:, :],
                             start=True, stop=True)
            s_row = smp.tile([1, br], F32, tag="s_row")
            nc.scalar.copy(s_row[:, :], ps_s[0:1, :])

            # R1: AllReduce branch normalizers
            cc1_in = drp.tile([1, br], F32, tag="cc1_in")
            cc1_out = drp.tile([1, br], F32, tag="cc1_out")
            nc.sync.dma_start(out=cc1_in[:, :], in_=s_row[:, :])
            nc.gpsimd.collective_compute(
                "AllReduce", mybir.AluOpType.add, replica_groups=rg,
                ins=[cc1_in[:, :].opt()], outs=[cc1_out[:, :].opt()],
            )
            sg = smp.tile([1, br], F32, tag="sg")
            nc.sync.dma_start(out=sg[:, :], in_=cc1_out[:, :])
            if NPAD_L > 0:
                nc.vector.tensor_scalar_add(sg[:, :], sg[:, :],
                                            float(-NPAD_L * cores))

            # w[n] = sum_br e[n,br] / (4*S_br)   -> [128, T]
            rs = smp.tile([1, br], F32, tag="rs")
            nc.vector.reciprocal(rs[:, :], sg[:, :])
            nc.vector.tensor_scalar_mul(rs[:, :], rs[:, :], 1.0 / br)
            rs_bc = smp.tile([P, br], F32, tag="rs_bc")
            nc.gpsimd.partition_broadcast(rs_bc[:, :], rs[:, :])
            w4 = smp.tile([P, T * br], F32, tag="w4")
            e3 = e_sb[:, :].rearrange("p (t b) -> p t b", b=br)
            w43 = w4[:, :].rearrange("p (t b) -> p t b", b=br)
            for j in range(br):
                nc.vector.tensor_scalar(
                    w43[:, :, j], e3[:, :, j], rs_bc[:, j:j + 1], None,
                    mybir.AluOpType.mult,
                )
            w_sb = smp.tile([P, T], F32, tag="w_sb")
            nc.vector.tensor_reduce(w_sb[:, :], w43, axis=mybir.AxisListType.X,
                                    op=mybir.AluOpType.add)

            # ---------------- top-k threshold ----------------
            t8 = smp.tile([P, 8], F32, tag="t8")
            nc.vector.max(t8[:, :], w_sb[:, :])
            t8_dr = drp.tile([P, 8], F32, tag="t8_dr")
            nc.sync.dma_start(out=t8_dr[:, :], in_=t8[:, :])
            cand = smp.tile([1, P * 8], F32, tag="cand")
            nc.sync.dma_start(out=cand[:, :],
                              in_=t8_dr[:, :].rearrange("p e -> (p e)"))
            c16 = smp.tile([1, 16], F32, tag="c16")
            nc.vector.max(c16[:, 0:8], cand[:, :])
            cand2 = smp.tile([1, P * 8], F32, tag="cand2")
            nc.vector.match_replace(cand2[:, :], c16[:, 0:8], cand[:, :],
                                    -1e30)
            nc.vector.max(c16[:, 8:16], cand2[:, :])

            # R2: AllGather per-shard top-16
            cc2_in = drp.tile([1, 16], F32, tag="cc2_in")
            cc2_out = drp.tile([1, 16 * cores], F32, tag="cc2_out")
            nc.sync.dma_start(out=cc2_in[:, :], in_=c16[:, :])
            nc.gpsimd.collective_compute(
                "AllGather", mybir.AluOpType.bypass, replica_groups=rg,
                ins=[cc2_in[:, :].opt()], outs=[cc2_out[:, :].opt()],
            )
            g_sb = smp.tile([1, 16 * cores], F32, tag="g_sb")
            nc.sync.dma_start(out=g_sb[:, :], in_=cc2_out[:, :])
            g8a = smp.tile([1, 8], F32, tag="g8a")
            nc.vector.max(g8a[:, :], g_sb[:, :])
            g_sb2 = smp.tile([1, 16 * cores], F32, tag="g_sb2")
            nc.vector.match_replace(g_sb2[:, :], g8a[:, :], g_sb[:, :], -1e30)
            g8b = smp.tile([1, 8], F32, tag="g8b")
            nc.vector.max(g8b[:, :], g_sb2[:, :])
            # threshold = global rank-(topk) value
            assert 8 < topk <= 16
            thr = g8b[:, topk - 9: topk - 8]
            thr_bc = smp.tile([P, 1], F32, tag="thr_bc")
            nc.gpsimd.partition_broadcast(thr_bc[:, :], thr)

            # bag0 = sum_n f[n,:] -- independent of the collectives; emitted
            # here so PE runs it during the R1/R2 latency windows.
            for t in range(T):
                for h0 in range(0, d, 512):
                    h1 = min(h0 + 512, d)
                    nc.tensor.matmul(
                        ps_bag[0:1, h0:h1],
                        lhsT=ones_bf[:, :],
                        rhs=fb[t][:, h0:h1],
                        start=(t == 0), stop=False,
                        skip_group_check=True,
                    )

            # ---------------- mask, second softmax numerators ----------------
            wm = smp.tile([P, T], F32, tag="wm")
            nc.vector.scalar_tensor_tensor(
                wm[:, :], w_sb[:, :], thr_bc[:, 0:1], w_sb[:, :],
                op0=mybir.AluOpType.is_lt, op1=mybir.AluOpType.mult,
            )
            u1 = smp.tile([P, T], F32, tag="u1")
            dsum = smp.tile([P, 1], F32, tag="dsum")
            nc.scalar.activation(u1[:, :], wm[:, :],
                                 mybir.ActivationFunctionType.Exp,
                                 accum_out=dsum[:, :])
            up = smp.tile([P, T], F32, tag="up")      # u' = exp(wm) - 1
            nc.vector.tensor_scalar_add(up[:, :], u1[:, :], -1.0)
            up_bf = smp.tile([P, T], BF16, tag="up_bf")
            nc.vector.tensor_copy(up_bf[:, :], up[:, :])

            ps_d = psp.tile([1, 1], F32, tag="ps_d")
            nc.tensor.matmul(ps_d[0:1, :], lhsT=ones_f32[:, :],
                             rhs=dsum[:, :], start=True, stop=True)

            # pass B: accumulate sum_n u'[n] * f[n, :] into the same psum
            for t in range(T):
                for h0 in range(0, d, 512):
                    h1 = min(h0 + 512, d)
                    nc.tensor.matmul(
                        ps_bag[0:1, h0:h1],
                        lhsT=up_bf[:, t:t + 1],
                        rhs=fb[t][:, h0:h1],
                        start=False, stop=(t == T - 1),
                        skip_group_check=True,
                    )

            # R3: AllReduce [bag0+bag1 | denom]
            bagd = smp.tile([1, d + 1], F32, tag="bagd")
            nc.scalar.copy(bagd[:, 0:d], ps_bag[0:1, :])
            nc.scalar.copy(bagd[:, d:d + 1], ps_d[0:1, :])
            cc3_in = drp.tile([1, d + 1], F32, tag="cc3_in")
            cc3_out = drp.tile([1, d + 1], F32, tag="cc3_out")
            nc.sync.dma_start(out=cc3_in[:, :], in_=bagd[:, :])
            nc.gpsimd.collective_compute(
                "AllReduce", mybir.AluOpType.add, replica_groups=rg,
                ins=[cc3_in[:, :].opt()], outs=[cc3_out[:, :].opt()],
            )
            gb = smp.tile([1, d + 1], F32, tag="gb")
            nc.sync.dma_start(out=gb[:, :], in_=cc3_out[:, :])

            # ---------------- finals ----------------
            den = smp.tile([1, 1], F32, tag="den")
            nc.vector.tensor_scalar_add(den[:, :], gb[:, d:d + 1],
                                        float(-NPAD_L * cores))
            rden = smp.tile([1, 1], F32, tag="rden")
            nc.vector.reciprocal(rden[:, :], den[:, :])
            bag_o = smp.tile([1, d], F32, tag="bag_o")
            nc.vector.tensor_scalar(bag_o[:, :], gb[:, 0:d], rden[:, 0:1],
                                    None, mybir.AluOpType.mult)
            nc.sync.dma_start(out=out_bag, in_=bag_o[:, :])

            rden_bc = smp.tile([P, 1], F32, tag="rden_bc")
            nc.gpsimd.partition_broadcast(rden_bc[:, :], rden[:, :])
            w2 = smp.tile([P, T], F32, tag="w2")
            nc.vector.tensor_scalar(w2[:, :], u1[:, :], rden_bc[:, 0:1], None,
                                    mybir.AluOpType.mult)
            ps_w2t = psp.tile([T, P], F32, tag="ps_w2t")
            nc.tensor.matmul(ps_w2t[:, :], lhsT=w2[:, :], rhs=ident[:, :],
                             is_transpose=True, start=True, stop=True)
            w2t = smp.tile([T, P], F32, tag="w2t")
            nc.vector.tensor_copy(w2t[:, :], ps_w2t[:, :])
            nc.sync.dma_start(
                out=out_w[0:(T - 1) * P].rearrange("(t p) -> t p", p=P),
                in_=w2t[0:T - 1, :],
            )
            nc.sync.dma_start(
                out=out_w[(T - 1) * P: ns],
                in_=w2t[T - 1: T, 0:PROWS],
            )

    nc.compile()
    return nc


_NC_CACHE = {}


def _get_nc():
    if "nc" not in _NC_CACHE:
        _NC_CACHE["nc"] = build_nc()
    return _NC_CACHE["nc"]


def make_in_maps(features, W):
    wt = np.ascontiguousarray(W.T).astype(ml_dtypes.bfloat16)
    return [
        {"features": np.ascontiguousarray(features[c * NS:(c + 1) * NS]),
         "wt": wt}
        for c in range(CORES)
    ]


def kernel(features, W, b=None, **_ignored):
    features = np.asarray(features, dtype=np.float32)
    W = np.asarray(W, dtype=np.float32)
    nc = _get_nc()
    res = run_bass_kernel_spmd(nc, make_in_maps(features, W),
                               core_ids=list(range(CORES)))
    results = res.results
    bag = np.asarray(results[0]["out_bag"], dtype=np.float32)
    w = np.concatenate(
        [np.asarray(results[c]["out_w"], dtype=np.float32)
         for c in range(CORES)]
    )
    return bag, w


if __name__ == "__main__":
    nc = build_nc()
    print("build+compile OK;",
          sum(len(bb.instructions) for bb in nc.main_func.blocks),
          "instructions")


# revision 15
# speedup vs baseline: 2.4311x; 1.1675x over previous
"""ACMIL top-k masking kernel for 8 TRN2 NeuronCores.

Reference computation (N=50000, D=1024, BRANCHES=4, TOP_K=10):
    scores = features @ W.T + b          # [N, 4]   (b cancels in softmax)
    weights = softmax(scores, axis=0)    # over instances
    w = weights.mean(axis=1)             # [N]
    w[top_k(w, 10)] = 0
    w = softmax(w, axis=0)
    bag = w @ features                   # [D]
    returns (bag, w)

Distribution: shard instances (dim 0) across 8 cores (6250 rows each).
AllReduce the per-branch softmax normalizer, global top-k via per-shard
top-16 + AllGather + local rank-10 threshold, AllReduce the final pooled
bag + second-softmax denominator.
"""

import sys

for _p in ("/opt/trn_rl_repo",):
    if _p not in sys.path:
        sys.path.insert(0, _p)

import numpy as np
import ml_dtypes

import concourse.bass as bass
import concourse.bacc as bacc
import concourse.mybir as mybir
import concourse.tile as tile
from concourse.bass_utils import run_bass_kernel_spmd

F32 = mybir.dt.float32
BF16 = mybir.dt.bfloat16
I32 = mybir.dt.int32

N, D, BR, TOPK, CORES = 50000, 1024, 4, 10, 8
NS = N // CORES  # 6250 rows per core


def build_nc(ns=NS, d=D, br=BR, cores=CORES, use_dma_cast=False,
             warmup_cc=True):
    """Build the per-core Bass graph (SPMD: same graph on all cores)."""
    P = 128
    T = (ns + P - 1) // P          # n-tiles per shard
    PROWS = ns - (T - 1) * P       # real rows in last tile
    NPAD_L = T * P - ns            # pad rows per core
    DC = d // P                    # d-chunks
    topk = TOPK

    nc = bacc.Bacc("TRN2", target_bir_lowering=False, debug=False,
                   num_devices=cores)

    feats = nc.dram_tensor("features", [ns, d], F32, kind="ExternalInput").ap()
    # host passes W.T pre-cast to bf16 (4KB, layout prep only)
    wt_in = nc.dram_tensor("wt", [d, br], BF16, kind="ExternalInput").ap()
    out_w = nc.dram_tensor("out_w", [ns], F32, kind="ExternalOutput").ap()
    out_bag = nc.dram_tensor("out_bag", [d], F32, kind="ExternalOutput").ap()

    rg = [list(range(cores))]

    with tile.TileContext(nc) as tc:
        with (
            tc.tile_pool(name="fb", bufs=1) as fbp,       # resident bf16 tiles
            tc.tile_pool(name="ft", bufs=6) as ftp,       # fT stream
            tc.tile_pool(name="sm", bufs=1) as smp,       # small persistents
            tc.tile_pool(name="ps", bufs=1, space="PSUM") as psp,
            tc.tile_pool(name="dr", bufs=1, space="DRAM") as drp,
        ):
            # ---------------- setup ----------------
            wt_sb = smp.tile([P, DC * br], BF16, tag="wt_sb")
            nc.sync.dma_start(
                out=wt_sb[:, :].rearrange("p (c b) -> p c b", b=br),
                in_=wt_in.rearrange("(c p) b -> p c b", p=P),
            )
            ones_bf = smp.tile([P, 1], BF16, tag="ones_bf")
            nc.gpsimd.memset(ones_bf[:, :], 1.0)
            ones_f32 = smp.tile([P, 1], F32, tag="ones_f32")
            nc.gpsimd.memset(ones_f32[:, :], 1.0)

            # identity matrix for PE transpose of the output weights
            iota_j = smp.tile([P, P], I32, tag="iota_j")
            nc.gpsimd.iota(iota_j[:, :], pattern=[[1, P]], base=0,
                           channel_multiplier=0)
            iota_p = smp.tile([P, 1], I32, tag="iota_p")
            nc.gpsimd.iota(iota_p[:, :], pattern=[[0, 1]], base=0,
                           channel_multiplier=1)
            iota_jf = smp.tile([P, P], F32, tag="iota_jf")
            nc.vector.tensor_copy(iota_jf[:, :], iota_j[:, :])
            iota_pf = smp.tile([P, 1], F32, tag="iota_pf")
            nc.vector.tensor_copy(iota_pf[:, :], iota_p[:, :])
            ident = smp.tile([P, P], F32, tag="ident")
            nc.vector.tensor_scalar(ident[:, :], iota_jf[:, :],
                                    iota_pf[:, :], None,
                                    mybir.AluOpType.is_equal)
            ident_bf = smp.tile([P, P], BF16, tag="ident_bf")
            nc.vector.tensor_copy(ident_bf[:, :], ident[:, :])

            if warmup_cc:
                wu_in = drp.tile([1, br], F32, tag="wu_in")
                wu_out = drp.tile([1, br], F32, tag="wu_out")
                nc.sync.dma_start(out=wu_in[:, :],
                                  in_=ident[0:1, 0:br])
                nc.gpsimd.collective_compute(
                    "AllReduce", mybir.AluOpType.add, replica_groups=rg,
                    ins=[wu_in[:, :].opt()], outs=[wu_out[:, :].opt()],
                )

            # ---------------- phase A: load + scores + bag0 ----------------
            ps_sc = psp.tile([P, T * br], F32, tag="ps_sc")    # score accum
            # bag psum accumulates BOTH sum_n f (phase A) and
            # sum_n u'*f (pass B) -- one accumulation group.
            ps_bag = psp.tile([1, d], F32, tag="ps_bag")

            fb = []
            for t in range(T):
                fb_t = fbp.tile([P, d], BF16, tag=f"fb{t}")
                fb.append(fb_t)
                rows = PROWS if t == T - 1 else P
                if rows < P:
                    nc.gpsimd.memset(fb_t[:, :], 0.0)
                if use_dma_cast:
                    nc.gpsimd.dma_start(out=fb_t[0:rows, :],
                                        in_=feats[t * P: t * P + rows, :])
                else:
                    fs_t = ftp.tile([P, d], F32, tag="fs")
                    nc.sync.dma_start(out=fs_t[0:rows, :],
                                      in_=feats[t * P: t * P + rows, :])
                    if t % 3 == 1:
                        nc.scalar.copy(fb_t[0:rows, :], fs_t[0:rows, :])
                    else:
                        nc.vector.tensor_copy(fb_t[0:rows, :],
                                              fs_t[0:rows, :])

                # transpose on PE via identity matmul, one PSUM bank per tile
                ptr_t = psp.tile([P, d], BF16, tag="ptr", bufs=2)
                for c in range(DC):
                    nc.tensor.transpose(ptr_t[:, c * P:(c + 1) * P],
                                        fb_t[:, c * P:(c + 1) * P],
                                        ident_bf[:, :])
                ft_t = ftp.tile([P, d], BF16, tag="ft")
                if t % 3 == 0:
                    nc.scalar.copy(ft_t[:, :], ptr_t[:, :])
                else:
                    nc.vector.tensor_copy(ft_t[:, :], ptr_t[:, :])
                for c in range(DC):
                    nc.tensor.matmul(
                        ps_sc[:, t * br:(t + 1) * br],
                        lhsT=ft_t[:, c * P:(c + 1) * P],
                        rhs=wt_sb[:, c * br:(c + 1) * br],
                        start=(c == 0), stop=(c == DC - 1),
                    )

            # ---------------- e = exp(scores), branch sums ----------------
            e_sb = smp.tile([P, T * br], F32, tag="e_sb")
            nc.scalar.activation(e_sb[:, :], ps_sc[:, :],
                                 mybir.ActivationFunctionType.Exp)
            sp = smp.tile([P, br], F32, tag="sp")  # per-partition branch sums
            nc.vector.tensor_reduce(
                sp[:, :],
                e_sb[:, :].rearrange("p (t b) -> p b t", b=br),
                axis=mybir.AxisListType.X, op=mybir.AluOpType.add,
            )
            ps_s = psp.tile([1, br], F32, tag="ps_s")
            nc.tensor.matmul(ps_s[0:1, :], lhsT=ones_f32[:, :], rhs=sp[:, :],
                             start=True, stop=True)
            s_row = smp.tile([1, br], F32, tag="s_row")
            nc.scalar.copy(s_row[:, :], ps_s[0:1, :])

            # R1: AllReduce branch normalizers
            cc1_in = drp.tile([1, br], F32, tag="cc1_in")
            cc1_out = drp.tile([1, br], F32, tag="cc1_out")
            nc.sync.dma_start(out=cc1_in[:, :], in_=s_row[:, :])
            nc.gpsimd.collective_compute(
                "AllReduce", mybir.AluOpType.add, replica_groups=rg,
                ins=[cc1_in[:, :].opt()], outs=[cc1_out[:, :].opt()],
            )
            sg = smp.tile([1, br], F32, tag="sg")
            nc.sync.dma_start(out=sg[:, :], in_=cc1_out[:, :])
            if NPAD_L > 0:
                nc.vector.tensor_scalar_add(sg[:, :], sg[:, :],
                                            float(-NPAD_L * cores))

            # w[n] = sum_br e[n,br] / (4*S_br)   -> [128, T]
            rs = smp.tile([1, br], F32, tag="rs")
            nc.vector.reciprocal(rs[:, :], sg[:, :])
            nc.vector.tensor_scalar_mul(rs[:, :], rs[:, :], 1.0 / br)
            rs_bc = smp.tile([P, br], F32, tag="rs_bc")
            nc.gpsimd.partition_broadcast(rs_bc[:, :], rs[:, :])
            w4 = smp.tile([P, T * br], F32, tag="w4")
            e3 = e_sb[:, :].rearrange("p (t b) -> p t b", b=br)
            w43 = w4[:, :].rearrange("p (t b) -> p t b", b=br)
            for j in range(br):
                nc.vector.tensor_scalar(
                    w43[:, :, j], e3[:, :, j], rs_bc[:, j:j + 1], None,
                    mybir.AluOpType.mult,
                )
            w_sb = smp.tile([P, T], F32, tag="w_sb")
            nc.vector.tensor_reduce(w_sb[:, :], w43, axis=mybir.AxisListType.X,
                                    op=mybir.AluOpType.add)

            # ---------------- top-k threshold ----------------
            t8 = smp.tile([P, 8], F32, tag="t8")
            nc.vector.max(t8[:, :], w_sb[:, :])
            t8_dr = drp.tile([P, 8], F32, tag="t8_dr")
            nc.sync.dma_start(out=t8_dr[:, :], in_=t8[:, :])
            cand = smp.tile([1, P * 8], F32, tag="cand")
            nc.sync.dma_start(out=cand[:, :],
                              in_=t8_dr[:, :].rearrange("p e -> (p e)"))
            c16 = smp.tile([1, 16], F32, tag="c16")
            nc.vector.max(c16[:, 0:8], cand[:, :])
            cand2 = smp.tile([1, P * 8], F32, tag="cand2")
            nc.vector.match_replace(cand2[:, :], c16[:, 0:8], cand[:, :],
                                    -1e30)
            nc.vector.max(c16[:, 8:16], cand2[:, :])

            # R2: AllGather per-shard top-16
            cc2_in = drp.tile([1, 16], F32, tag="cc2_in")
            cc2_out = drp.tile([1, 16 * cores], F32, tag="cc2_out")
            nc.sync.dma_start(out=cc2_in[:, :], in_=c16[:, :])
            nc.gpsimd.collective_compute(
                "AllGather", mybir.AluOpType.bypass, replica_groups=rg,
                ins=[cc2_in[:, :].opt()], outs=[cc2_out[:, :].opt()],
            )
            g_sb = smp.tile([1, 16 * cores], F32, tag="g_sb")
            nc.sync.dma_start(out=g_sb[:, :], in_=cc2_out[:, :])
            g8a = smp.tile([1, 8], F32, tag="g8a")
            nc.vector.max(g8a[:, :], g_sb[:, :])
            g_sb2 = smp.tile([1, 16 * cores], F32, tag="g_sb2")
            nc.vector.match_replace(g_sb2[:, :], g8a[:, :], g_sb[:, :], -1e30)
            g8b = smp.tile([1, 8], F32, tag="g8b")
            nc.vector.max(g8b[:, :], g_sb2[:, :])
            # threshold = global rank-(topk) value
            assert 8 < topk <= 16
            thr = g8b[:, topk - 9: topk - 8]
            thr_bc = smp.tile([P, 1], F32, tag="thr_bc")
            nc.gpsimd.partition_broadcast(thr_bc[:, :], thr)

            # bag0 = sum_n f[n,:] -- independent of the collectives; emitted
            # here so PE runs it during the R1/R2 latency windows.
            for t in range(T):
                for h0 in range(0, d, 512):
                    h1 = min(h0 + 512, d)
                    nc.tensor.matmul(
                        ps_bag[0:1, h0:h1],
                        lhsT=ones_bf[:, :],
                        rhs=fb[t][:, h0:h1],
                        start=(t == 0), stop=False,
                        skip_group_check=True,
                    )

            # ---------------- mask, second softmax numerators ----------------
            wm = smp.tile([P, T], F32, tag="wm")
            nc.vector.scalar_tensor_tensor(
                wm[:, :], w_sb[:, :], thr_bc[:, 0:1], w_sb[:, :],
                op0=mybir.AluOpType.is_lt, op1=mybir.AluOpType.mult,
            )
            u1 = smp.tile([P, T], F32, tag="u1")
            dsum = smp.tile([P, 1], F32, tag="dsum")
            nc.scalar.activation(u1[:, :], wm[:, :],
                                 mybir.ActivationFunctionType.Exp,
                                 accum_out=dsum[:, :])
            up = smp.tile([P, T], F32, tag="up")      # u' = exp(wm) - 1
            nc.vector.tensor_scalar_add(up[:, :], u1[:, :], -1.0)
            up_bf = smp.tile([P, T], BF16, tag="up_bf")
            nc.vector.tensor_copy(up_bf[:, :], up[:, :])

            ps_d = psp.tile([1, 1], F32, tag="ps_d")
            nc.tensor.matmul(ps_d[0:1, :], lhsT=ones_f32[:, :],
                             rhs=dsum[:, :], start=True, stop=True)

            # transpose u1 now so only a tiny scale remains after R3
            ps_w2t = psp.tile([T, P], F32, tag="ps_w2t")
            nc.tensor.matmul(ps_w2t[:, :], lhsT=u1[:, :], rhs=ident[:, :],
                             is_transpose=True, start=True, stop=True)
            u1t = smp.tile([T, P], F32, tag="u1t")
            nc.vector.tensor_copy(u1t[:, :], ps_w2t[:, :])

            # pass B: accumulate sum_n u'[n] * f[n, :] into the same psum
            for t in range(T):
                for h0 in range(0, d, 512):
                    h1 = min(h0 + 512, d)
                    nc.tensor.matmul(
                        ps_bag[0:1, h0:h1],
                        lhsT=up_bf[:, t:t + 1],
                        rhs=fb[t][:, h0:h1],
                        start=False, stop=(t == T - 1),
                        skip_group_check=True,
                    )

            # R3: AllReduce [bag0+bag1 | denom]
            bagd = smp.tile([1, d + 1], F32, tag="bagd")
            nc.scalar.copy(bagd[:, 0:d], ps_bag[0:1, :])
            nc.scalar.copy(bagd[:, d:d + 1], ps_d[0:1, :])
            cc3_in = drp.tile([1, d + 1], F32, tag="cc3_in")
            cc3_out = drp.tile([1, d + 1], F32, tag="cc3_out")
            nc.sync.dma_start(out=cc3_in[:, :], in_=bagd[:, :])
            nc.gpsimd.collective_compute(
                "AllReduce", mybir.AluOpType.add, replica_groups=rg,
                ins=[cc3_in[:, :].opt()], outs=[cc3_out[:, :].opt()],
            )
            gb = smp.tile([1, d + 1], F32, tag="gb")
            nc.sync.dma_start(out=gb[:, :], in_=cc3_out[:, :])

            # ---------------- finals ----------------
            den = smp.tile([1, 1], F32, tag="den")
            nc.vector.tensor_scalar_add(den[:, :], gb[:, d:d + 1],
                                        float(-NPAD_L * cores))
            rden = smp.tile([1, 1], F32, tag="rden")
            nc.vector.reciprocal(rden[:, :], den[:, :])
            bag_o = smp.tile([1, d], F32, tag="bag_o")
            nc.vector.tensor_scalar(bag_o[:, :], gb[:, 0:d], rden[:, 0:1],
                                    None, mybir.AluOpType.mult)
            nc.sync.dma_start(out=out_bag, in_=bag_o[:, :])

            rden_bc = smp.tile([P, 1], F32, tag="rden_bc")
            nc.gpsimd.partition_broadcast(rden_bc[:, :], rden[:, :])
            w2t = smp.tile([T, P], F32, tag="w2t")
            nc.vector.tensor_scalar(w2t[:, :], u1t[:, :], rden_bc[0:T, 0:1],
                                    None, mybir.AluOpType.mult)
            nc.sync.dma_start(
                out=out_w[0:(T - 1) * P].rearrange("(t p) -> t p", p=P),
                in_=w2t[0:T - 1, :],
            )
            nc.sync.dma_start(
                out=out_w[(T - 1) * P: ns],
                in_=w2t[T - 1: T, 0:PROWS],
            )

    nc.compile()
    return nc


_NC_CACHE = {}


def _get_nc():
    if "nc" not in _NC_CACHE:
        _NC_CACHE["nc"] = build_nc()
    return _NC_CACHE["nc"]


def make_in_maps(features, W):
    wt = np.ascontiguousarray(W.T).astype(ml_dtypes.bfloat16)
    return [
        {"features": np.ascontiguousarray(features[c * NS:(c + 1) * NS]),
         "wt": wt}
        for c in range(CORES)
    ]


def kernel(features, W, b=None, **_ignored):
    features = np.asarray(features, dtype=np.float32)
    W = np.asarray(W, dtype=np.float32)
    nc = _get_nc()
    res = run_bass_kernel_spmd(nc, make_in_maps(features, W),
                               core_ids=list(range(CORES)))
    results = res.results
    bag = np.asarray(results[0]["out_bag"], dtype=np.float32)
    w = np.concatenate(
        [np.asarray(results[c]["out_w"], dtype=np.float32)
         for c in range(CORES)]
    )
    return bag, w


if __name__ == "__main__":
    nc = build_nc()
    print("build+compile OK;",
          sum(len(bb.instructions) for bb in nc.main_func.blocks),
          "instructions")
